# revision 35
# baseline (speedup 1.0000x reference)
"""AdaptiveCompressionLayer kernel for 8 TRN2 NeuronCores.

Strategy (expert-routed data parallel):
  - Host: bucket tokens by importance score (>0.8 / >0.4 / rest), gather
    tokens into per-expert groups, split evenly across 8 cores with fixed
    per-expert capacities (multiples of 512), and pre-transpose the routed
    activations to [H, T_pad] so the device needs no on-chip transposes.
  - Device (SPMD, identical graph on all 8 cores): for each 512-token
    group (single expert per group, known at compile time):
        Z^T = Wc^T @ X^T   (PE, bf16, f32 PSUM accumulation)
        Z^T += bc          (ScalarE activation copy w/ per-partition bias)
        Y   = Z^T.T @ [Wd; bd]  (ones-row trick folds bd into the matmul)
        out = LayerNorm(Y) (bn_stats/bn_aggr + sqrt/recip + affine)
  - Host: scatter valid rows back to the original token order.

No cross-core communication: routing is per-token, weights replicated.
"""
import sys

sys.path.insert(0, "/opt/trn_rl_repo")

import numpy as np
import ml_dtypes

BF16 = ml_dtypes.bfloat16

H = 768
HC = (691, 537, 76)
S = 65536
EPS = 1e-5
N_CORES = 8
GROUP = 512
# Per-core, per-expert token capacities (multiples of GROUP).
# Expected per-core counts for uniform scores: ~1638 / ~3277 / ~3277.
CAPS = (1792, 3328, 3328)  # default; kernel() tightens from actual counts
TPAD = sum(CAPS)  # 8448
OFFS = (0, CAPS[0], CAPS[0] + CAPS[1])
# chunk counts along hc (mm1 M-chunks == mm2 K-chunks; bias row fits in last)
MC = tuple((hc + 127) // 128 for hc in HC)  # (6, 5, 1)

TRACE = False
LAST_RESULT = None

_NC_CACHE = {}


def _weave(caps):
    per_e = []
    for e in range(3):
        offs = (0, caps[0], caps[0] + caps[1])
        glist = []
        t = 0
        while t < caps[e]:
            gsz = min(GROUP, caps[e] - t)
            glist.append((e, offs[e] + t, gsz))
            t += gsz
        per_e.append(glist)
    woven = []
    credit = [0.0, 0.0, 0.0]
    total = sum(len(g) for g in per_e)
    first_uses = []
    for _ in range(total):
        for e in range(3):
            if per_e[e]:
                credit[e] += len(per_e[e])
        order = sorted(range(3), key=lambda e: -credit[e])
        pick = order[0]
        # the last expert to first-appear gets its weights DMA'd last;
        # don't schedule it in the first 4 groups
        if len(woven) < 4 and pick not in first_uses and len(first_uses) >= 2:
            for alt in order[1:]:
                if per_e[alt] and alt in first_uses:
                    pick = alt
                    break
        if not per_e[pick]:
            pick = next(e for e in order if per_e[e])
        credit[pick] = 0.0
        if pick not in first_uses:
            first_uses.append(pick)
        woven.append(per_e[pick].pop(0))
    # shortest group last -> minimal post-matmul LN tail
    smallest = min(range(len(woven)), key=lambda i: woven[i][2])
    woven.append(woven.pop(smallest))
    return woven


def _weave_first_use_order(caps):
    seen = []
    for e, _, _ in _weave(caps):
        if e not in seen:
            seen.append(e)
    return seen


def _build(apply_gb: bool, caps=CAPS):
    import concourse.bass as bass
    import concourse.mybir as mybir
    import concourse.tile as tile
    from concourse import bacc

    f32 = mybir.dt.float32
    bf16 = mybir.dt.bfloat16
    AF = mybir.ActivationFunctionType
    ALU = mybir.AluOpType

    tpad = sum(caps)
    offs = (0, caps[0], caps[0] + caps[1])

    nc = bacc.Bacc(None, target_bir_lowering=False)

    xt_d = nc.declare_dram_parameter("xt", [H, tpad], bf16, isOutput=False)
    wc_d = [
        nc.declare_dram_parameter(f"wc{e}", [H, HC[e]], bf16, isOutput=False)
        for e in range(3)
    ]
    wdb_d = [
        nc.declare_dram_parameter(f"wdb{e}", [HC[e] + 1, H], bf16, isOutput=False)
        for e in range(3)
    ]
    bcp_d = nc.declare_dram_parameter("bcp", [128, 18], f32, isOutput=False)
    if apply_gb:
        gb_d = nc.declare_dram_parameter("gb", [2, H], f32, isOutput=False)
    out_d = nc.declare_dram_parameter("out", [tpad, H], f32, isOutput=True)

    with tile.TileContext(nc) as tc:
        from contextlib import ExitStack

        with ExitStack() as ctx:
            wpool = ctx.enter_context(tc.tile_pool(name="weights", bufs=1))
            cpool = ctx.enter_context(tc.tile_pool(name="consts", bufs=1))
            xpool = ctx.enter_context(tc.tile_pool(name="xt", bufs=6))
            zpsum = ctx.enter_context(tc.tile_pool(name="zpsum", bufs=2, space="PSUM"))
            zpool = ctx.enter_context(tc.tile_pool(name="zsb", bufs=4))
            ypsum = ctx.enter_context(tc.tile_pool(name="ypsum", bufs=3, space="PSUM"))
            opool = ctx.enter_context(tc.tile_pool(name="osb", bufs=8))
            lnpool = ctx.enter_context(tc.tile_pool(name="ln", bufs=8))

            # ---- constants first (tiny; first z-copy needs bc) ----
            bc_sb = cpool.tile([128, 18], f32)
            nc.scalar.dma_start(out=bc_sb, in_=bcp_d[:, :])
            eps_t = cpool.tile([128, 1], f32)
            nc.vector.memset(eps_t, EPS)
            if apply_gb:
                gb_sb = cpool.tile([128, 2, H], f32)
                nc.scalar.dma_start(
                    out=gb_sb,
                    in_=gb_d.ap().partition_broadcast(128),
                )

            # ---- weight tiles; DMA issues are interleaved into the first
            # groups of the weave (sync queue) so they don't block the ACT
            # queue or the first xt load ----
            wc_sb = [None] * 3
            wd_sb = [None] * 3
            for e in range(3):
                wc_sb[e] = wpool.tile([128, 6, HC[e]], bf16, tag=f"wc{e}", name=f"wc_sb{e}")
                wd_sb[e] = wpool.tile([128, MC[e], H], bf16, tag=f"wd{e}", name=f"wd_sb{e}")

            def _issue_wc(e):
                nc.sync.dma_start(
                    out=wc_sb[e],
                    in_=wc_d[e].ap().rearrange("(c p) h -> p c h", p=128),
                )

            def _issue_wd(e):
                hc = HC[e]
                for k in range(MC[e]):
                    rows = min(128, hc + 1 - k * 128)
                    nc.sync.dma_start(
                        out=wd_sb[e][0:rows, k, :],
                        in_=wdb_d[e][k * 128 : k * 128 + rows, :],
                    )

            def _issue_weights(e):
                _issue_wc(e)
                _issue_wd(e)

            pending_weights = [
                (lambda e=_e: _issue_weights(e))
                for _e in _weave_first_use_order(caps)
            ]

            # PE warm-up: dummy matmuls during the initial weight DMA wait
            # keep the HAM activity window hot so real matmuls start at
            # full clock.
            warm = cpool.tile([128, 512], bf16, name="warm")
            nc.vector.memset(warm, 0.0)
            warm_ps = zpsum.tile([128, 512], f32, tag="pz", name="warm_ps")
            for _w in range(14):
                nc.tensor.matmul(
                    warm_ps,
                    lhsT=warm[:, 0:128],
                    rhs=warm,
                    start=(_w == 0),
                    stop=(_w == 13),
                )
            xt_r = xt_d.ap().rearrange("(c p) t -> p c t", p=128)

            # ---- main loop over token groups (512s plus a 256 tail) ----
            # Interleave experts so PE-light (e2) groups overlap PE-heavy
            # ones on the other engines.
            woven = _weave(caps)
            subtile_no = 0
            for e, tok0, gsz in woven:
                hc = HC[e]
                mc = MC[e]
                orow = hc - 128 * (mc - 1)  # ones row within last chunk
                if True:
                    xt_t = xpool.tile([128, 6, gsz], bf16, tag="xt")
                    nc.sync.dma_start(
                        out=xt_t, in_=xt_r[:, :, tok0 : tok0 + gsz]
                    )
                    if pending_weights:
                        pending_weights.pop(0)()
                    zt = zpool.tile([128, 6, gsz], bf16, tag="zt")
                    # ones row for the bd term: memset a 32-aligned window
                    # covering partition `orow` of the last chunk; the ACT
                    # copy below overwrites the real z rows inside it.
                    w0 = (orow // 32) * 32
                    nc.gpsimd.memset(zt[w0 : w0 + 32, mc - 1, :], 1.0)
                    for m in range(mc):
                        hcm = min(128, hc - m * 128)
                        pz = zpsum.tile([128, gsz], f32, tag="pz")
                        for c in range(6):
                            nc.tensor.matmul(
                                pz[0:hcm, :],
                                lhsT=wc_sb[e][:, c, m * 128 : m * 128 + hcm],
                                rhs=xt_t[:, c, :],
                                start=(c == 0),
                                stop=(c == 5),
                            )
                        # Z += bc, PSUM -> SBUF on ScalarE
                        nc.scalar.activation(
                            out=zt[0:hcm, m, :],
                            in_=pz[0:hcm, :],
                            func=AF.Identity,
                            bias=bc_sb[0:hcm, e * 6 + m : e * 6 + m + 1],
                            scale=1.0,
                        )
                    for sub in range(gsz // 128):
                        py = ypsum.tile([128, H], f32, tag="py")
                        for k in range(mc):
                            kk = 128 if k < mc - 1 else orow + 1
                            for n0, nn in ((0, 512), (512, 256)):
                                nc.tensor.matmul(
                                    py[:, n0 : n0 + nn],
                                    lhsT=zt[0:kk, k, sub * 128 : (sub + 1) * 128],
                                    rhs=wd_sb[e][0:kk, k, n0 : n0 + nn],
                                    start=(k == 0),
                                    stop=(k == mc - 1),
                                )
                        # LayerNorm
                        stats = lnpool.tile([128, 2, 6], f32, tag="stats")
                        for j in range(2):
                            nc.vector.bn_stats(
                                out=stats[:, j, :], in_=py[:, j * 384 : (j + 1) * 384]
                            )
                        mv = lnpool.tile([128, 2], f32, tag="mv")
                        nc.vector.bn_aggr(out=mv, in_=stats)
                        rstd = lnpool.tile([128, 1], f32, tag="rstd")
                        nc.scalar.activation(
                            out=rstd,
                            in_=mv[:, 1:2],
                            func=AF.Sqrt,
                            bias=eps_t,
                            scale=1.0,
                        )
                        nc.vector.reciprocal(out=rstd, in_=rstd)
                        o_t = opool.tile([128, H], f32, tag="o")
                        subtile_no += 1
                        if subtile_no % 3 != 0:
                            negmu = lnpool.tile([128, 1], f32, tag="negmu")
                            nc.vector.tensor_scalar(
                                out=negmu,
                                in0=mv[:, 0:1],
                                scalar1=rstd[:, 0:1],
                                scalar2=-1.0,
                                op0=ALU.mult,
                                op1=ALU.mult,
                            )
                            nc.scalar.activation(
                                out=o_t,
                                in_=py,
                                func=AF.Identity,
                                bias=negmu,
                                scale=rstd[:, 0:1],
                            )
                        else:
                            nc.vector.tensor_scalar(
                                out=o_t,
                                in0=py,
                                scalar1=mv[:, 0:1],
                                scalar2=rstd[:, 0:1],
                                op0=ALU.subtract,
                                op1=ALU.mult,
                            )
                        if apply_gb:
                            nc.gpsimd.tensor_tensor(
                                out=o_t, in0=o_t, in1=gb_sb[:, 0, :], op=ALU.mult
                            )
                            nc.vector.tensor_add(o_t, o_t, gb_sb[:, 1, :])
                        nc.sync.dma_start(
                            out=out_d[tok0 + sub * 128 : tok0 + (sub + 1) * 128, :],
                            in_=o_t,
                        )
    nc.finalize()
    return nc


def _get_nc(apply_gb: bool, caps):
    key = (apply_gb, caps)
    if key not in _NC_CACHE:
        _NC_CACHE[key] = _build(apply_gb, caps=caps)
    return _NC_CACHE[key]


def kernel(**inputs):
    global LAST_RESULT
    from concourse.bass_utils import run_bass_kernel_spmd

    hs = np.ascontiguousarray(np.asarray(inputs["hidden_states"], dtype=np.float32))
    sc = np.asarray(inputs["importance_scores"], dtype=np.float32)
    gamma = np.asarray(inputs["gamma"], dtype=np.float32)
    beta = np.asarray(inputs["beta"], dtype=np.float32)

    # routing (must match f32 comparison semantics of the reference)
    m0 = sc > np.float32(0.8)
    m1 = (sc > np.float32(0.4)) & ~m0
    bucket = np.where(m0, 0, np.where(m1, 1, 2)).astype(np.int64)
    idx = [np.flatnonzero(bucket == e) for e in range(3)]
    splits = [np.array_split(idx[e], N_CORES) for e in range(3)]

    # tight per-core caps: max per-core count rounded up to 128
    caps = tuple(
        int(-(-max(len(p) for p in splits[e]) // 128) * 128) for e in range(3)
    )
    tpad = sum(caps)
    offs = (0, caps[0], caps[0] + caps[1])

    gidx = np.zeros((N_CORES, tpad), np.int64)
    valid = np.zeros((N_CORES, tpad), bool)
    for c in range(N_CORES):
        for e in range(3):
            p = splits[e][c]
            o = offs[e]
            gidx[c, o : o + len(p)] = p
            valid[c, o : o + len(p)] = True

    # packed bc: column e*6+m = bc_e[m*128 : m*128+128]
    bcp = np.zeros((128, 18), np.float32)
    for e in range(3):
        b = np.asarray(inputs[f"bc{e}"], dtype=np.float32)
        for m in range(MC[e]):
            seg = b[m * 128 : (m + 1) * 128]
            bcp[: len(seg), e * 6 + m] = seg

    apply_gb = not (np.all(gamma == 1.0) and np.all(beta == 0.0))
    nc = _get_nc(apply_gb, caps)

    base = {"bcp": bcp}
    for e in range(3):
        base[f"wc{e}"] = np.ascontiguousarray(
            np.asarray(inputs[f"Wc{e}"]).astype(BF16)
        )
        wd = np.asarray(inputs[f"Wd{e}"], dtype=np.float32)
        bd = np.asarray(inputs[f"bd{e}"], dtype=np.float32)
        base[f"wdb{e}"] = np.ascontiguousarray(
            np.concatenate([wd, bd[None, :]], axis=0).astype(BF16)
        )
    if apply_gb:
        base["gb"] = np.ascontiguousarray(np.stack([gamma, beta], axis=0))

    in_maps = []
    for c in range(N_CORES):
        xc = hs[gidx[c]]  # [TPAD, H]
        m = dict(base)
        m["xt"] = np.ascontiguousarray(xc.T.astype(BF16))
        in_maps.append(m)

    # The device occasionally returns corrupted (non-finite) results right
    # after an unrecoverable-state episode; inputs are finite and LayerNorm
    # output is always finite, so retry on any non-finite value.
    for attempt in range(3):
        res = run_bass_kernel_spmd(
            nc, in_maps, core_ids=list(range(N_CORES)), trace=TRACE
        )
        LAST_RESULT = res
        out = np.empty((S, H), np.float32)
        for c in range(N_CORES):
            v = valid[c]
            out[gidx[c][v]] = res.results[c]["out"][v]
        if np.isfinite(out).all():
            break
    return out
